# revision 23
# baseline (speedup 1.0000x reference)
"""CRF NLL loss kernel for Trainium2 (8 NeuronCores, batch-sharded).

Strategy (v2)
-------------
Data-parallel over batch: each of 8 cores handles BC=64 sequences.

Forward algorithm in the EXP DOMAIN with labels on partitions, batch on
the free dim: w_t[l, b] ~ exp(fv_t[l, b] - t*C0).  One step is a single
bf16 PE matmul with stationary Ep2 = exp(transitions - C0) plus one DVE
multiply by exp(features_t):

    w_t = ef_t * (Ep2^T @ w_{t-1})

Capture trick: labels PAD(0) and BOS(1) have identically-zero forward
mass under the CRF's constrained transitions, so column 0 of Ep2 is
replaced by texp = exp(trans[:, EOS]) (with texp[PAD/BOS] := 0) and row
0 of Ep2 is zeroed.  Then row 0 of every matmul output carries
z_{t-1} = sum_p exp(trans[p,EOS]) * w_{t-1}[p] -- the log-partition
numerator -- for free.  Host feature marshalling zeroes feature row 0
(so ef[0] = 1) and w_t[0] = z_{t-1} rides along in the state; every 16
steps the ring row 0 is DMA'd out, and the host selects z at t* = len-1
per sequence.

Rescaling: every 16 steps rc = 1/w[0] (the z row, bf16) is recorded and
applied OFF the critical path to a future emission tile (17 steps
later), broadcast across partitions with a 1-partition matmul.  The
host un-does the logged rc factors in log space (events with
s_app <= t*+1; the export step carries the factor applied at it).

Gold path score: host gathers the indexed scalars feat[b,t,tag] and
trans[tag,tag']; the device does the masked weighted sums.

All matmuls are bf16 (one PE pass instead of fp32's two); bf16 keeps
fp32's exponent range so the exp-domain state cannot over/underflow any
faster, and the loss tolerance (2e-2 relative on a ~1e5 loss) dwarfs
bf16 rounding.
"""

import numpy as np

B, T, L = 512, 512, 128
NCORES = 8
BC = B // NCORES            # 64 sequences per core
PAD, BOS, EOS = 0, 1, 2
C0 = 5.83                   # per-step log-shift folded into Ep2 (~mean drift)
CH = 8                      # steps per feature chunk
NCHUNK = T // CH            # 64 chunks
RING = 32                   # w ring slots
NEV = 15                    # rescale events: measured at t=31+32ev, applied at t=40+32ev
PREF = 2                    # chunks prefetched ahead
DUMMY_MM = 2                # idle-filling PE matmuls per step (keep clock ramped)

F32 = np.float32

_compiled = None


def _build():
    import concourse.bass as bass
    import concourse.bacc as bacc
    import concourse.mybir as mybir
    import concourse.tile as tile

    f32 = mybir.dt.float32
    bf16 = mybir.dt.bfloat16
    nc = bacc.Bacc("TRN2", target_bir_lowering=False, debug=False)

    featc = nc.dram_tensor("featc", [NCHUNK, L, CH * BC], f32, kind="ExternalInput")
    ep2 = nc.dram_tensor("ep2", [L, L], bf16, kind="ExternalInput")
    emis_v = nc.dram_tensor("emis_v", [BC, T], f32, kind="ExternalInput")
    emis_w = nc.dram_tensor("emis_w", [BC, T], f32, kind="ExternalInput")
    trans_v = nc.dram_tensor("trans_v", [BC, T + 1], f32, kind="ExternalInput")
    trans_w = nc.dram_tensor("trans_w", [BC, T + 1], f32, kind="ExternalInput")

    zrows_o = nc.dram_tensor("zrows", [T // 16, 16 * BC], bf16, kind="ExternalOutput")
    zlast_o = nc.dram_tensor("zlast", [1, BC], f32, kind="ExternalOutput")
    recips_o = nc.dram_tensor("recips", [1, NEV * BC], f32, kind="ExternalOutput")
    gold_o = nc.dram_tensor("gold", [BC, 1], f32, kind="ExternalOutput")

    AX = mybir.AxisListType.X
    MUL = mybir.AluOpType.mult
    ADD = mybir.AluOpType.add
    EXP = mybir.ActivationFunctionType.Exp

    with tile.TileContext(nc) as tc:
        with (
            tc.tile_pool(name="state", bufs=1) as st,
            tc.tile_pool(name="feat", bufs=PREF + 1) as fp,
            tc.tile_pool(name="ef", bufs=PREF + 1) as efp,
            tc.tile_pool(name="vpa", bufs=2, space="PSUM") as vpa,
            tc.tile_pool(name="vpb", bufs=2, space="PSUM") as vpb,
            tc.tile_pool(name="bcps", bufs=1, space="PSUM") as bcps,
            tc.tile_pool(name="zps", bufs=1, space="PSUM") as zps,
            tc.tile_pool(name="dps", bufs=1, space="PSUM") as dps,
            tc.tile_pool(name="misc", bufs=1) as mp,
        ):
            # ---- chunk prep helper ----
            ef_tiles = {}

            def prep_chunk(c):
                if c >= NCHUNK:
                    return
                ft = fp.tile([L, CH * BC], f32, tag="ftile")
                nc.sync.dma_start(ft[:], featc[c])
                ef = efp.tile([L, CH * BC], bf16, tag="ef")
                nc.scalar.activation(ef[:], ft[:], EXP, bias=0.0, scale=1.0)
                ef_tiles[c] = ef

            prep_chunk(0)

            # ---- one-time setup ----
            ep2_sb = st.tile([L, L], bf16)
            nc.sync.dma_start(ep2_sb[:], ep2[:])
            ones_row = st.tile([1, L], f32)     # lhsT for partition broadcast
            nc.vector.memset(ones_row[:], 1.0)

            wring = st.tile([L, RING * BC], bf16)
            recips = st.tile([1, NEV * BC], f32)
            dmy = dps.tile([1, BC // 2], f32, space="PSUM")

            for c in range(1, PREF + 1):
                prep_chunk(c)

            # ---- init: w_0 = ef_0[:, 0:BC] (BOS row folded into feat t=0) ----
            nc.vector.tensor_copy(wring[:, 0:BC], ef_tiles[0][:, 0:BC])

            # ---- recurrence over t = 1..T-1 ----
            for t in range(1, T):
                c, j = t // CH, t % CH
                s, sp = (t % RING) * BC, ((t - 1) % RING) * BC
                if j == 0:
                    prep_chunk(c + PREF)
                    del ef_tiles[c - 1]
                    # rescale application onto this chunk's first block
                    if c >= 5 and (c - 5) % 4 == 0 and (c - 5) // 4 < NEV:
                        ev = (c - 5) // 4
                        bc_ps = bcps.tile([L, BC], f32, space="PSUM")
                        nc.tensor.matmul(bc_ps[:], lhsT=ones_row[:],
                                         rhs=recips[:, ev * BC:(ev + 1) * BC],
                                         start=True, stop=True)
                        efc = ef_tiles[c]
                        nc.vector.tensor_tensor(out=efc[:, 0:BC], in0=bc_ps[:],
                                                in1=efc[:, 0:BC], op=MUL)

                # two column groups, software-pipelined so PE and DVE overlap
                HB = BC // 2
                efc = ef_tiles[c]
                va = vpa.tile([L, HB], f32, space="PSUM")
                nc.tensor.matmul(va[:], lhsT=ep2_sb[:],
                                 rhs=wring[:, sp:sp + HB], start=True, stop=True)
                vb = vpb.tile([L, HB], f32, space="PSUM")
                nc.tensor.matmul(vb[:], lhsT=ep2_sb[:],
                                 rhs=wring[:, sp + HB:sp + BC],
                                 start=True, stop=True)
                nc.vector.tensor_tensor(out=wring[:, s:s + HB], in0=va[:],
                                        in1=efc[:, j * BC:j * BC + HB], op=MUL)
                nc.vector.tensor_tensor(out=wring[:, s + HB:s + BC], in0=vb[:],
                                        in1=efc[:, j * BC + HB:(j + 1) * BC],
                                        op=MUL)

                if t % 32 == 31:
                    # record rescale reciprocal from the z row (fp32 PSUM);
                    # approximate is fine -- the factor is logged and un-done
                    # exactly on the host
                    ev = (t - 31) // 32
                    if ev < NEV:
                        nc.vector.reciprocal_approx_fast(
                            recips[:, ev * BC:ev * BC + HB], va[0:1, :])
                        nc.vector.reciprocal_approx_fast(
                            recips[:, ev * BC + HB:(ev + 1) * BC], vb[0:1, :])
                if t % 16 == 15:
                    # export z rows (16 slots ending at slot of t)
                    w = (t - 15) // 16
                    lo = ((t - 15) % RING) * BC
                    nc.sync.dma_start(zrows_o[w:w + 1, :],
                                      wring[0:1, lo:lo + 16 * BC])

                if DUMMY_MM:
                    # idle-filling matmuls keep the PE busy so the clock stays
                    # at full p-state; results are never read
                    for _ in range(DUMMY_MM):
                        nc.tensor.matmul(dmy[:], lhsT=ep2_sb[:, 0:1],
                                         rhs=ep2_sb[:, 0:HB],
                                         start=True, stop=True,
                                         skip_group_check=True)

            # ---- final z_{T-1}: one more (1-col) matmul ----
            vz = zps.tile([1, BC], f32, space="PSUM")
            sl = ((T - 1) % RING) * BC
            nc.tensor.matmul(vz[:], lhsT=ep2_sb[:, 0:1], rhs=wring[:, sl:sl + BC],
                             start=True, stop=True)
            zl = mp.tile([1, BC], f32, tag="zl")
            nc.vector.tensor_copy(zl[:], vz[:])
            nc.sync.dma_start(zlast_o[:], zl[:])
            nc.sync.dma_start(recips_o[:], recips[:])

            # ---- gold score masked sums ----
            ev_sb = mp.tile([BC, T], f32, tag="gv")
            nc.sync.dma_start(ev_sb[:], emis_v[:])
            ew_sb = mp.tile([BC, T], f32, tag="gw")
            nc.sync.dma_start(ew_sb[:], emis_w[:])
            nc.vector.tensor_tensor(out=ev_sb[:], in0=ev_sb[:], in1=ew_sb[:], op=MUL)
            g1 = mp.tile([BC, 1], f32, tag="g1")
            nc.vector.reduce_sum(g1[:], ev_sb[:], axis=AX)

            tv_sb = mp.tile([BC, T + 1], f32, tag="tv")
            nc.sync.dma_start(tv_sb[:], trans_v[:])
            tw_sb = mp.tile([BC, T + 1], f32, tag="tw")
            nc.sync.dma_start(tw_sb[:], trans_w[:])
            nc.vector.tensor_tensor(out=tv_sb[:], in0=tv_sb[:], in1=tw_sb[:], op=MUL)
            g2 = mp.tile([BC, 1], f32, tag="g2")
            nc.vector.reduce_sum(g2[:], tv_sb[:], axis=AX)
            nc.vector.tensor_tensor(out=g1[:], in0=g1[:], in1=g2[:], op=ADD)
            nc.sync.dma_start(gold_o[:], g1[:])

    nc.compile()
    return nc


def _get_compiled():
    global _compiled
    if _compiled is None:
        _compiled = _build()
    return _compiled


def _host_consts(trans_np):
    import ml_dtypes

    Ep = np.exp(trans_np.astype(np.float64) - C0)
    texp = np.exp(trans_np[:, EOS].astype(np.float64))
    texp[PAD] = 0.0
    texp[BOS] = 0.0
    Ep2 = Ep.copy()
    Ep2[:, PAD] = texp            # output col 0 carries z
    Ep2[PAD, :] = 0.0             # z-row garbage leaks nowhere
    return np.ascontiguousarray(Ep2.astype(ml_dtypes.bfloat16))


def _prep_core(feat, tags, maskf, trans_np, ep2_bf16):
    """Host-side marshalling for one core's shard."""
    featm = feat.copy()
    featm[:, 0, :] += trans_np[BOS, :][None, :]
    featm[:, :, PAD] = 0.0        # ef row 0 == 1 -> w[0] = z passthrough
    fc = featm.transpose(1, 2, 0)                             # [T, L, BC]
    fc = fc.reshape(NCHUNK, CH, L, BC).transpose(0, 2, 1, 3)  # [NCHUNK,L,CH,BC]
    featc = np.ascontiguousarray(fc.reshape(NCHUNK, L, CH * BC))

    lens = maskf.sum(axis=1).astype(np.int64)
    tstar = lens - 1

    emis_v = np.take_along_axis(feat, tags[..., None], axis=-1)[..., 0]  # [BC,T]
    emis_w = maskf.copy()
    emis_w[:, 0] = 1.0

    trans_v = np.empty((BC, T + 1), dtype=F32)
    trans_v[:, : T - 1] = trans_np[tags[:, :-1], tags[:, 1:]]
    trans_v[:, T - 1] = trans_np[BOS, tags[:, 0]]
    last_lab = tags[np.arange(BC), tstar]
    trans_v[:, T] = trans_np[last_lab, EOS]
    trans_w = np.empty((BC, T + 1), dtype=F32)
    trans_w[:, : T - 1] = maskf[:, 1:]
    trans_w[:, T - 1] = 1.0
    trans_w[:, T] = 1.0

    in_map = {
        "featc": featc,
        "ep2": ep2_bf16,
        "emis_v": np.ascontiguousarray(emis_v.astype(F32)),
        "emis_w": np.ascontiguousarray(emis_w),
        "trans_v": trans_v,
        "trans_w": trans_w,
    }
    return in_map, tstar


def _prep_all(inputs):
    feats = np.asarray(inputs["features"], dtype=F32)
    tags = np.asarray(inputs["tag_seqs"])
    maskf = np.asarray(inputs["mask"]).astype(F32)
    trans_np = np.asarray(inputs["transitions"], dtype=F32)
    ep2_bf16 = _host_consts(trans_np)
    in_maps = []
    for c in range(NCORES):
        sl = slice(c * BC, (c + 1) * BC)
        m, _ = _prep_core(feats[sl], tags[sl], maskf[sl], trans_np, ep2_bf16)
        in_maps.append(m)
    return in_maps


def kernel(features, tag_seqs, mask, transitions):
    from concourse import bass_utils

    feats = np.asarray(features, dtype=F32)
    tags = np.asarray(tag_seqs)
    maskf = np.asarray(mask).astype(F32)
    trans_np = np.asarray(transitions, dtype=F32)

    nc = _get_compiled()
    ep2_bf16 = _host_consts(trans_np)

    in_maps, tstars = [], []
    for c in range(NCORES):
        sl = slice(c * BC, (c + 1) * BC)
        m, ts = _prep_core(feats[sl], tags[sl], maskf[sl], trans_np, ep2_bf16)
        in_maps.append(m)
        tstars.append(ts)

    res = bass_utils.run_bass_kernel_spmd(nc, in_maps, core_ids=list(range(NCORES)))

    s_app = 40 + 32 * np.arange(NEV)               # event ev applied at step s_app
    per_seq = []
    for c in range(NCORES):
        out = res.results[c]
        ts = tstars[c]                              # [BC]
        zr = np.asarray(out["zrows"]).astype(np.float64).reshape(T // 16, 16, BC)
        zlast = np.asarray(out["zlast"]).astype(np.float64)[0]
        rc = np.asarray(out["recips"]).astype(np.float64).reshape(NEV, BC)
        te = ts + 1                                 # export step of z_{t*}
        bidx = np.arange(BC)
        z_sel = np.where(te >= T, zlast, zr[np.minimum(te // 16, T // 16 - 1),
                                           te % 16, bidx])
        applies = s_app[:, None] <= te[None, :]     # export step carries its factor
        logcorr = (-np.log(rc) * applies).sum(axis=0)
        logZ = np.log(z_sel) + ts * C0 + logcorr
        gold = np.asarray(out["gold"]).astype(np.float64)[:, 0]
        per_seq.append(gold - logZ)

    loss = -np.mean(np.concatenate(per_seq))
    return np.float32(loss)


# revision 26
# speedup vs baseline: 1.3061x; 1.3061x over previous
"""CRF NLL loss kernel for Trainium2 (8 NeuronCores, batch-sharded).

Strategy (v2)
-------------
Data-parallel over batch: each of 8 cores handles BC=64 sequences.

Forward algorithm in the EXP DOMAIN with labels on partitions, batch on
the free dim: w_t[l, b] ~ exp(fv_t[l, b] - t*C0).  One step is a single
bf16 PE matmul with stationary Ep2 = exp(transitions - C0) plus one DVE
multiply by exp(features_t):

    w_t = ef_t * (Ep2^T @ w_{t-1})

Capture trick: labels PAD(0) and BOS(1) have identically-zero forward
mass under the CRF's constrained transitions, so column 0 of Ep2 is
replaced by texp = exp(trans[:, EOS]) (with texp[PAD/BOS] := 0) and row
0 of Ep2 is zeroed.  Then row 0 of every matmul output carries
z_{t-1} = sum_p exp(trans[p,EOS]) * w_{t-1}[p] -- the log-partition
numerator -- for free.  Host feature marshalling zeroes feature row 0
(so ef[0] = 1) and w_t[0] = z_{t-1} rides along in the state; every 16
steps the ring row 0 is DMA'd out, and the host selects z at t* = len-1
per sequence.

Rescaling: every 16 steps rc = 1/w[0] (the z row, bf16) is recorded and
applied OFF the critical path to a future emission tile (17 steps
later), broadcast across partitions with a 1-partition matmul.  The
host un-does the logged rc factors in log space (events with
s_app <= t*+1; the export step carries the factor applied at it).

Gold path score: host gathers the indexed scalars feat[b,t,tag] and
trans[tag,tag']; the device does the masked weighted sums.

All matmuls are bf16 (one PE pass instead of fp32's two); bf16 keeps
fp32's exponent range so the exp-domain state cannot over/underflow any
faster, and the loss tolerance (2e-2 relative on a ~1e5 loss) dwarfs
bf16 rounding.
"""

import numpy as np

B, T, L = 512, 512, 128
NCORES = 8
BC = B // NCORES            # 64 sequences per core
PAD, BOS, EOS = 0, 1, 2
C0 = 5.83                   # per-step log-shift folded into Ep2 (~mean drift)
CH = 8                      # steps per feature chunk
NCHUNK = T // CH            # 64 chunks
RING = 32                   # w ring slots
NEV = 15                    # rescale events: measured at t=31+32ev, applied at t=40+32ev
PREF = 2                    # chunks prefetched ahead
DUMMY_MM = 0                # idle-filling PE matmuls per step (tested: hurts)

F32 = np.float32

_compiled = None


def _build():
    import concourse.bass as bass
    import concourse.bacc as bacc
    import concourse.mybir as mybir
    import concourse.tile as tile

    f32 = mybir.dt.float32
    bf16 = mybir.dt.bfloat16
    nc = bacc.Bacc("TRN2", target_bir_lowering=False, debug=False)

    featc = nc.dram_tensor("featc", [NCHUNK, L, CH * BC], f32, kind="ExternalInput")
    ep2 = nc.dram_tensor("ep2", [L, L], bf16, kind="ExternalInput")
    emis_v = nc.dram_tensor("emis_v", [BC, T], f32, kind="ExternalInput")
    emis_w = nc.dram_tensor("emis_w", [BC, T], f32, kind="ExternalInput")
    trans_v = nc.dram_tensor("trans_v", [BC, T + 1], f32, kind="ExternalInput")
    trans_w = nc.dram_tensor("trans_w", [BC, T + 1], f32, kind="ExternalInput")

    zrows_o = nc.dram_tensor("zrows", [T // 16, 16 * BC], bf16, kind="ExternalOutput")
    zlast_o = nc.dram_tensor("zlast", [1, BC], f32, kind="ExternalOutput")
    recips_o = nc.dram_tensor("recips", [1, NEV * BC], f32, kind="ExternalOutput")
    gold_o = nc.dram_tensor("gold", [BC, 1], f32, kind="ExternalOutput")

    AX = mybir.AxisListType.X
    MUL = mybir.AluOpType.mult
    ADD = mybir.AluOpType.add
    EXP = mybir.ActivationFunctionType.Exp

    with tile.TileContext(nc) as tc:
        with (
            tc.tile_pool(name="state", bufs=1) as st,
            tc.tile_pool(name="feat", bufs=PREF + 1) as fp,
            tc.tile_pool(name="ef", bufs=PREF + 1) as efp,
            tc.tile_pool(name="vpa", bufs=2, space="PSUM") as vpa,
            tc.tile_pool(name="vpb", bufs=2, space="PSUM") as vpb,
            tc.tile_pool(name="bcps", bufs=1, space="PSUM") as bcps,
            tc.tile_pool(name="zps", bufs=1, space="PSUM") as zps,
            tc.tile_pool(name="dps", bufs=1, space="PSUM") as dps,
            tc.tile_pool(name="misc", bufs=1) as mp,
        ):
            # ---- chunk prep helper ----
            ef_tiles = {}

            def prep_chunk(c):
                if c >= NCHUNK:
                    return
                ft = fp.tile([L, CH * BC], f32, tag="ftile")
                nc.sync.dma_start(ft[:], featc[c])
                ef = efp.tile([L, CH * BC], bf16, tag="ef")
                nc.scalar.activation(ef[:], ft[:], EXP, bias=0.0, scale=1.0)
                ef_tiles[c] = ef

            prep_chunk(0)

            # ---- one-time setup ----
            ep2_sb = st.tile([L, L], bf16)
            nc.sync.dma_start(ep2_sb[:], ep2[:])
            ones_row = st.tile([1, L], f32)     # lhsT for partition broadcast
            nc.vector.memset(ones_row[:], 1.0)

            wring = st.tile([L, RING * BC], bf16)
            recips = st.tile([1, NEV * BC], f32)
            dmy = dps.tile([1, BC // 2], f32, space="PSUM")

            for c in range(1, PREF + 1):
                prep_chunk(c)

            # ---- gold score masked sums (overlaps kernel startup) ----
            ev_sb = mp.tile([BC, T], f32, tag="gv")
            nc.sync.dma_start(ev_sb[:], emis_v[:])
            ew_sb = mp.tile([BC, T], f32, tag="gw")
            nc.sync.dma_start(ew_sb[:], emis_w[:])
            nc.vector.tensor_tensor(out=ev_sb[:], in0=ev_sb[:], in1=ew_sb[:], op=MUL)
            g1 = mp.tile([BC, 1], f32, tag="g1")
            nc.vector.reduce_sum(g1[:], ev_sb[:], axis=AX)

            tv_sb = mp.tile([BC, T + 1], f32, tag="tv")
            nc.sync.dma_start(tv_sb[:], trans_v[:])
            tw_sb = mp.tile([BC, T + 1], f32, tag="tw")
            nc.sync.dma_start(tw_sb[:], trans_w[:])
            nc.vector.tensor_tensor(out=tv_sb[:], in0=tv_sb[:], in1=tw_sb[:], op=MUL)
            g2 = mp.tile([BC, 1], f32, tag="g2")
            nc.vector.reduce_sum(g2[:], tv_sb[:], axis=AX)
            nc.vector.tensor_tensor(out=g1[:], in0=g1[:], in1=g2[:], op=ADD)
            nc.sync.dma_start(gold_o[:], g1[:])

            # ---- init: w_0 = ef_0[:, 0:BC] (BOS row folded into feat t=0) ----
            nc.vector.tensor_copy(wring[:, 0:BC], ef_tiles[0][:, 0:BC])

            # ---- recurrence over t = 1..T-1 ----
            for t in range(1, T):
                c, j = t // CH, t % CH
                s, sp = (t % RING) * BC, ((t - 1) % RING) * BC
                if j == 0:
                    prep_chunk(c + PREF)
                    del ef_tiles[c - 1]
                    # rescale application onto this chunk's first block
                    if c >= 5 and (c - 5) % 4 == 0 and (c - 5) // 4 < NEV:
                        ev = (c - 5) // 4
                        bc_ps = bcps.tile([L, BC], f32, space="PSUM")
                        nc.tensor.matmul(bc_ps[:], lhsT=ones_row[:],
                                         rhs=recips[:, ev * BC:(ev + 1) * BC],
                                         start=True, stop=True)
                        efc = ef_tiles[c]
                        nc.vector.tensor_tensor(out=efc[:, 0:BC], in0=bc_ps[:],
                                                in1=efc[:, 0:BC], op=MUL)

                # two column groups, software-pipelined so PE and DVE overlap
                HB = BC // 2
                efc = ef_tiles[c]
                va = vpa.tile([L, HB], f32, space="PSUM")
                nc.tensor.matmul(va[:], lhsT=ep2_sb[:],
                                 rhs=wring[:, sp:sp + HB], start=True, stop=True)
                vb = vpb.tile([L, HB], f32, space="PSUM")
                nc.tensor.matmul(vb[:], lhsT=ep2_sb[:],
                                 rhs=wring[:, sp + HB:sp + BC],
                                 start=True, stop=True)
                nc.vector.tensor_tensor(out=wring[:, s:s + HB], in0=va[:],
                                        in1=efc[:, j * BC:j * BC + HB], op=MUL)
                nc.vector.tensor_tensor(out=wring[:, s + HB:s + BC], in0=vb[:],
                                        in1=efc[:, j * BC + HB:(j + 1) * BC],
                                        op=MUL)

                if t % 32 == 31:
                    # record rescale reciprocal from the z row (fp32 PSUM);
                    # approximate is fine -- the factor is logged and un-done
                    # exactly on the host
                    ev = (t - 31) // 32
                    if ev < NEV:
                        nc.vector.reciprocal_approx_fast(
                            recips[:, ev * BC:ev * BC + HB], va[0:1, :])
                        nc.vector.reciprocal_approx_fast(
                            recips[:, ev * BC + HB:(ev + 1) * BC], vb[0:1, :])
                if t % 16 == 15:
                    # export z rows (16 slots ending at slot of t)
                    w = (t - 15) // 16
                    lo = ((t - 15) % RING) * BC
                    nc.sync.dma_start(zrows_o[w:w + 1, :],
                                      wring[0:1, lo:lo + 16 * BC])

                if DUMMY_MM:
                    # idle-filling matmuls keep the PE busy so the clock stays
                    # at full p-state; results are never read
                    for _ in range(DUMMY_MM):
                        nc.tensor.matmul(dmy[:], lhsT=ep2_sb[:, 0:1],
                                         rhs=ep2_sb[:, 0:HB],
                                         start=True, stop=True,
                                         skip_group_check=True)

            # ---- final z_{T-1}: one more (1-col) matmul ----
            vz = zps.tile([1, BC], f32, space="PSUM")
            sl = ((T - 1) % RING) * BC
            nc.tensor.matmul(vz[:], lhsT=ep2_sb[:, 0:1], rhs=wring[:, sl:sl + BC],
                             start=True, stop=True)
            zl = mp.tile([1, BC], f32, tag="zl")
            nc.vector.tensor_copy(zl[:], vz[:])
            nc.sync.dma_start(zlast_o[:], zl[:])
            nc.sync.dma_start(recips_o[:], recips[:])

    nc.compile()
    return nc


def _get_compiled():
    global _compiled
    if _compiled is None:
        _compiled = _build()
    return _compiled


def _host_consts(trans_np):
    import ml_dtypes

    Ep = np.exp(trans_np.astype(np.float64) - C0)
    texp = np.exp(trans_np[:, EOS].astype(np.float64))
    texp[PAD] = 0.0
    texp[BOS] = 0.0
    Ep2 = Ep.copy()
    Ep2[:, PAD] = texp            # output col 0 carries z
    Ep2[PAD, :] = 0.0             # z-row garbage leaks nowhere
    return np.ascontiguousarray(Ep2.astype(ml_dtypes.bfloat16))


def _prep_core(feat, tags, maskf, trans_np, ep2_bf16):
    """Host-side marshalling for one core's shard."""
    featm = feat.copy()
    featm[:, 0, :] += trans_np[BOS, :][None, :]
    featm[:, :, PAD] = 0.0        # ef row 0 == 1 -> w[0] = z passthrough
    fc = featm.transpose(1, 2, 0)                             # [T, L, BC]
    fc = fc.reshape(NCHUNK, CH, L, BC).transpose(0, 2, 1, 3)  # [NCHUNK,L,CH,BC]
    featc = np.ascontiguousarray(fc.reshape(NCHUNK, L, CH * BC))

    lens = maskf.sum(axis=1).astype(np.int64)
    tstar = lens - 1

    emis_v = np.take_along_axis(feat, tags[..., None], axis=-1)[..., 0]  # [BC,T]
    emis_w = maskf.copy()
    emis_w[:, 0] = 1.0

    trans_v = np.empty((BC, T + 1), dtype=F32)
    trans_v[:, : T - 1] = trans_np[tags[:, :-1], tags[:, 1:]]
    trans_v[:, T - 1] = trans_np[BOS, tags[:, 0]]
    last_lab = tags[np.arange(BC), tstar]
    trans_v[:, T] = trans_np[last_lab, EOS]
    trans_w = np.empty((BC, T + 1), dtype=F32)
    trans_w[:, : T - 1] = maskf[:, 1:]
    trans_w[:, T - 1] = 1.0
    trans_w[:, T] = 1.0

    in_map = {
        "featc": featc,
        "ep2": ep2_bf16,
        "emis_v": np.ascontiguousarray(emis_v.astype(F32)),
        "emis_w": np.ascontiguousarray(emis_w),
        "trans_v": trans_v,
        "trans_w": trans_w,
    }
    return in_map, tstar


def _prep_all(inputs):
    feats = np.asarray(inputs["features"], dtype=F32)
    tags = np.asarray(inputs["tag_seqs"])
    maskf = np.asarray(inputs["mask"]).astype(F32)
    trans_np = np.asarray(inputs["transitions"], dtype=F32)
    ep2_bf16 = _host_consts(trans_np)
    in_maps = []
    for c in range(NCORES):
        sl = slice(c * BC, (c + 1) * BC)
        m, _ = _prep_core(feats[sl], tags[sl], maskf[sl], trans_np, ep2_bf16)
        in_maps.append(m)
    return in_maps


def kernel(features, tag_seqs, mask, transitions):
    from concourse import bass_utils

    feats = np.asarray(features, dtype=F32)
    tags = np.asarray(tag_seqs)
    maskf = np.asarray(mask).astype(F32)
    trans_np = np.asarray(transitions, dtype=F32)

    nc = _get_compiled()
    ep2_bf16 = _host_consts(trans_np)

    in_maps, tstars = [], []
    for c in range(NCORES):
        sl = slice(c * BC, (c + 1) * BC)
        m, ts = _prep_core(feats[sl], tags[sl], maskf[sl], trans_np, ep2_bf16)
        in_maps.append(m)
        tstars.append(ts)

    res = bass_utils.run_bass_kernel_spmd(nc, in_maps, core_ids=list(range(NCORES)))

    s_app = 40 + 32 * np.arange(NEV)               # event ev applied at step s_app
    per_seq = []
    for c in range(NCORES):
        out = res.results[c]
        ts = tstars[c]                              # [BC]
        zr = np.asarray(out["zrows"]).astype(np.float64).reshape(T // 16, 16, BC)
        zlast = np.asarray(out["zlast"]).astype(np.float64)[0]
        rc = np.asarray(out["recips"]).astype(np.float64).reshape(NEV, BC)
        te = ts + 1                                 # export step of z_{t*}
        bidx = np.arange(BC)
        z_sel = np.where(te >= T, zlast, zr[np.minimum(te // 16, T // 16 - 1),
                                           te % 16, bidx])
        applies = s_app[:, None] <= te[None, :]     # export step carries its factor
        logcorr = (-np.log(rc) * applies).sum(axis=0)
        logZ = np.log(z_sel) + ts * C0 + logcorr
        gold = np.asarray(out["gold"]).astype(np.float64)[:, 0]
        per_seq.append(gold - logZ)

    loss = -np.mean(np.concatenate(per_seq))
    return np.float32(loss)


# revision 40
# speedup vs baseline: 1.3392x; 1.0253x over previous
"""CRF NLL loss kernel for Trainium2 (8 NeuronCores, batch-sharded).

Strategy
--------
Data-parallel over batch: each of 8 cores handles BC=64 sequences.
The forward algorithm is a latency-bound sequential chain (511 dependent
matmul->multiply roundtrips), so the design minimizes per-step latency.

Forward algorithm in the EXP DOMAIN with labels on partitions, batch on
the free dim: w_t[l, b] ~ exp(fv_t[l, b] - t*C0).  One step is a single
bf16 PE matmul with stationary Ep2 = exp(transitions - C0) plus one DVE
multiply by exp(features_t):

    w_t = ef_t * (Ep2^T @ w_{t-1})

The batch is split into two 32-column groups software-pipelined so the
PE matmul of one group overlaps the DVE multiply of the other (measured
step ~467ns vs ~527ns unsplit).

Capture trick: labels PAD(0) and BOS(1) have identically-zero forward
mass under the CRF's constrained transitions, so column 0 of Ep2 is
replaced by texp = exp(trans[:, EOS]) (with texp[PAD/BOS] := 0) and row
0 of Ep2 is zeroed.  Then row 0 of every matmul output carries
z_{t-1} = sum_p exp(trans[p,EOS]) * w_{t-1}[p] -- the log-partition
numerator -- for free.  Host feature marshalling zeroes feature row 0
(so ef[0] = 1) and w_t[0] = z_{t-1} rides along in the state; every 16
steps the ring row 0 is DMA'd out, and the host selects z at
t* = len-1 per sequence.

Rescaling (fp32/bf16 range control): every 32 steps the z row (ring
slot 31, also exported) is broadcast across partitions with a
1-partition bf16 matmul, reciprocal'd (approx), and multiplied into a
future emission tile -- all off the critical chain.  The host un-does
exactly these factors in log space using the exported bf16 z values
(events with s_app <= t*+1; the export step carries the factor applied
at it).

Gold path score: host gathers the indexed scalars feat[b,t,tag] and
trans[tag,tag']; the device does the masked weighted sums.

All matmuls are bf16 (one PE pass instead of fp32's two); bf16 keeps
fp32's exponent range so the exp-domain state cannot over/underflow any
faster, and the loss tolerance (2e-2 relative on a ~1e5 loss) dwarfs
bf16 rounding.
"""

import numpy as np

B, T, L = 512, 512, 128
NCORES = 8
BC = B // NCORES            # 64 sequences per core
PAD, BOS, EOS = 0, 1, 2
C0 = 5.83                   # per-step log-shift folded into Ep2 (~mean drift)
CH = 8                      # steps per feature chunk
NCHUNK = T // CH            # 64 chunks
RING = 32                   # w ring slots
NEV = 15                    # rescale events: measured at t=31+32ev, applied at t=40+32ev
PREF = 2                    # chunks prefetched ahead
DUMMY_MM = 0                # idle-filling PE matmuls per step (tested: hurts)

F32 = np.float32

_compiled = None


def _build():
    import concourse.bass as bass
    import concourse.bacc as bacc
    import concourse.mybir as mybir
    import concourse.tile as tile

    f32 = mybir.dt.float32
    bf16 = mybir.dt.bfloat16
    nc = bacc.Bacc("TRN2", target_bir_lowering=False, debug=False)

    featc = nc.dram_tensor("featc", [NCHUNK, L, CH * BC], f32, kind="ExternalInput")
    ep2 = nc.dram_tensor("ep2", [L, L], bf16, kind="ExternalInput")
    emis_v = nc.dram_tensor("emis_v", [BC, T], f32, kind="ExternalInput")
    emis_w = nc.dram_tensor("emis_w", [BC, T], f32, kind="ExternalInput")
    trans_v = nc.dram_tensor("trans_v", [BC, T + 1], f32, kind="ExternalInput")
    trans_w = nc.dram_tensor("trans_w", [BC, T + 1], f32, kind="ExternalInput")

    zrows_o = nc.dram_tensor("zrows", [T // 16, 16 * BC], bf16, kind="ExternalOutput")
    zlast_o = nc.dram_tensor("zlast", [1, BC], f32, kind="ExternalOutput")
    gold_o = nc.dram_tensor("gold", [BC, 1], f32, kind="ExternalOutput")

    AX = mybir.AxisListType.X
    MUL = mybir.AluOpType.mult
    ADD = mybir.AluOpType.add
    DIV = mybir.AluOpType.divide
    EXP = mybir.ActivationFunctionType.Exp

    with tile.TileContext(nc) as tc:
        with (
            tc.tile_pool(name="state", bufs=1) as st,
            tc.tile_pool(name="feat", bufs=PREF + 1) as fp,
            tc.tile_pool(name="ef", bufs=PREF + 1) as efp,
            tc.tile_pool(name="vpa", bufs=3, space="PSUM") as vpa,
            tc.tile_pool(name="vpb", bufs=3, space="PSUM") as vpb,
            tc.tile_pool(name="bcps", bufs=1, space="PSUM") as bcps,
            tc.tile_pool(name="zps", bufs=1, space="PSUM") as zps,
            tc.tile_pool(name="misc", bufs=1) as mp,
        ):
            # ---- chunk prep helper ----
            ef_tiles = {}

            def prep_chunk(c):
                if c >= NCHUNK:
                    return
                ft = fp.tile([L, CH * BC], f32, tag="ftile")
                nc.sync.dma_start(ft[:], featc[c])
                ef = efp.tile([L, CH * BC], bf16, tag="ef")
                nc.scalar.activation(ef[:], ft[:], EXP, bias=0.0, scale=1.0)
                ef_tiles[c] = ef

            prep_chunk(0)

            # ---- one-time setup ----
            ep2_sb = st.tile([L, L], bf16)
            nc.sync.dma_start(ep2_sb[:], ep2[:])
            ones_row = st.tile([1, L], bf16)    # lhsT for partition broadcast
            nc.vector.memset(ones_row[:], 1.0)

            wring = st.tile([L, RING * BC], bf16)
            bc_sb = st.tile([L, BC], f32)       # broadcast 1/z rescale factors

            for c in range(1, PREF + 1):
                prep_chunk(c)

            # ---- gold score masked sums (overlaps kernel startup) ----
            ev_sb = mp.tile([BC, T], f32, tag="gv")
            nc.sync.dma_start(ev_sb[:], emis_v[:])
            ew_sb = mp.tile([BC, T], f32, tag="gw")
            nc.sync.dma_start(ew_sb[:], emis_w[:])
            nc.vector.tensor_tensor(out=ev_sb[:], in0=ev_sb[:], in1=ew_sb[:], op=MUL)
            g1 = mp.tile([BC, 1], f32, tag="g1")
            nc.vector.reduce_sum(g1[:], ev_sb[:], axis=AX)

            tv_sb = mp.tile([BC, T + 1], f32, tag="tv")
            nc.sync.dma_start(tv_sb[:], trans_v[:])
            tw_sb = mp.tile([BC, T + 1], f32, tag="tw")
            nc.sync.dma_start(tw_sb[:], trans_w[:])
            nc.vector.tensor_tensor(out=tv_sb[:], in0=tv_sb[:], in1=tw_sb[:], op=MUL)
            g2 = mp.tile([BC, 1], f32, tag="g2")
            nc.vector.reduce_sum(g2[:], tv_sb[:], axis=AX)
            nc.vector.tensor_tensor(out=g1[:], in0=g1[:], in1=g2[:], op=ADD)
            nc.sync.dma_start(gold_o[:], g1[:])

            # ---- init: w_0 = ef_0[:, 0:BC] (BOS row folded into feat t=0) ----
            nc.vector.tensor_copy(wring[:, 0:BC], ef_tiles[0][:, 0:BC])

            # ---- recurrence over t = 1..T-1 ----
            for t in range(1, T):
                c, j = t // CH, t % CH
                s, sp = (t % RING) * BC, ((t - 1) % RING) * BC
                if j == 0:
                    prep_chunk(c + PREF)
                    del ef_tiles[c - 1]
                    # rescale application onto this chunk's first block:
                    # broadcast the z row measured 9 steps ago (ring slot 31),
                    # take its reciprocal, and scale the emission tile.  The
                    # same bf16 z is exported in zrows so the host un-does it
                    # (the tiny approx-reciprocal residual is far below the
                    # loss tolerance).
                    if c >= 5 and (c - 5) % 4 == 0 and (c - 5) // 4 < NEV:
                        bc_ps = bcps.tile([L, BC], f32, space="PSUM")
                        nc.tensor.matmul(bc_ps[:], lhsT=ones_row[:],
                                         rhs=wring[0:1, 31 * BC:32 * BC],
                                         start=True, stop=True)
                        nc.vector.reciprocal_approx_fast(bc_sb[:], bc_ps[:])
                        efc = ef_tiles[c]
                        nc.vector.tensor_tensor(out=efc[:, 0:BC],
                                                in0=efc[:, 0:BC],
                                                in1=bc_sb[:], op=MUL)

                # two column groups, software-pipelined so PE and DVE overlap
                HB = BC // 2
                efc = ef_tiles[c]
                va = vpa.tile([L, HB], f32, space="PSUM")
                nc.tensor.matmul(va[:], lhsT=ep2_sb[:],
                                 rhs=wring[:, sp:sp + HB], start=True, stop=True)
                vb = vpb.tile([L, HB], f32, space="PSUM")
                nc.tensor.matmul(vb[:], lhsT=ep2_sb[:],
                                 rhs=wring[:, sp + HB:sp + BC],
                                 start=True, stop=True)
                nc.vector.tensor_tensor(out=wring[:, s:s + HB], in0=va[:],
                                        in1=efc[:, j * BC:j * BC + HB], op=MUL)
                nc.vector.tensor_tensor(out=wring[:, s + HB:s + BC], in0=vb[:],
                                        in1=efc[:, j * BC + HB:(j + 1) * BC],
                                        op=MUL)

                if t % 16 == 15:
                    # export z rows (16 slots ending at slot of t)
                    w = (t - 15) // 16
                    lo = ((t - 15) % RING) * BC
                    nc.sync.dma_start(zrows_o[w:w + 1, :],
                                      wring[0:1, lo:lo + 16 * BC])

            # ---- final z_{T-1}: one more (1-col) matmul ----
            vz = zps.tile([1, BC], f32, space="PSUM")
            sl = ((T - 1) % RING) * BC
            nc.tensor.matmul(vz[:], lhsT=ep2_sb[:, 0:1], rhs=wring[:, sl:sl + BC],
                             start=True, stop=True)
            zl = mp.tile([1, BC], f32, tag="zl")
            nc.vector.tensor_copy(zl[:], vz[:])
            nc.sync.dma_start(zlast_o[:], zl[:])

    nc.compile()
    return nc


def _get_compiled():
    global _compiled
    if _compiled is None:
        _compiled = _build()
    return _compiled


def _host_consts(trans_np):
    import ml_dtypes

    Ep = np.exp(trans_np.astype(np.float64) - C0)
    texp = np.exp(trans_np[:, EOS].astype(np.float64))
    texp[PAD] = 0.0
    texp[BOS] = 0.0
    Ep2 = Ep.copy()
    Ep2[:, PAD] = texp            # output col 0 carries z
    Ep2[PAD, :] = 0.0             # z-row garbage leaks nowhere
    return np.ascontiguousarray(Ep2.astype(ml_dtypes.bfloat16))


def _prep_core(feat, tags, maskf, trans_np, ep2_bf16):
    """Host-side marshalling for one core's shard."""
    featm = feat.copy()
    featm[:, 0, :] += trans_np[BOS, :][None, :]
    featm[:, :, PAD] = 0.0        # ef row 0 == 1 -> w[0] = z passthrough
    fc = featm.transpose(1, 2, 0)                             # [T, L, BC]
    fc = fc.reshape(NCHUNK, CH, L, BC).transpose(0, 2, 1, 3)  # [NCHUNK,L,CH,BC]
    featc = np.ascontiguousarray(fc.reshape(NCHUNK, L, CH * BC))

    lens = maskf.sum(axis=1).astype(np.int64)
    tstar = lens - 1

    emis_v = np.take_along_axis(feat, tags[..., None], axis=-1)[..., 0]  # [BC,T]
    emis_w = maskf.copy()
    emis_w[:, 0] = 1.0

    trans_v = np.empty((BC, T + 1), dtype=F32)
    trans_v[:, : T - 1] = trans_np[tags[:, :-1], tags[:, 1:]]
    trans_v[:, T - 1] = trans_np[BOS, tags[:, 0]]
    last_lab = tags[np.arange(BC), tstar]
    trans_v[:, T] = trans_np[last_lab, EOS]
    trans_w = np.empty((BC, T + 1), dtype=F32)
    trans_w[:, : T - 1] = maskf[:, 1:]
    trans_w[:, T - 1] = 1.0
    trans_w[:, T] = 1.0

    in_map = {
        "featc": featc,
        "ep2": ep2_bf16,
        "emis_v": np.ascontiguousarray(emis_v.astype(F32)),
        "emis_w": np.ascontiguousarray(emis_w),
        "trans_v": trans_v,
        "trans_w": trans_w,
    }
    return in_map, tstar


def _prep_all(inputs):
    feats = np.asarray(inputs["features"], dtype=F32)
    tags = np.asarray(inputs["tag_seqs"])
    maskf = np.asarray(inputs["mask"]).astype(F32)
    trans_np = np.asarray(inputs["transitions"], dtype=F32)
    ep2_bf16 = _host_consts(trans_np)
    in_maps = []
    for c in range(NCORES):
        sl = slice(c * BC, (c + 1) * BC)
        m, _ = _prep_core(feats[sl], tags[sl], maskf[sl], trans_np, ep2_bf16)
        in_maps.append(m)
    return in_maps


def kernel(features, tag_seqs, mask, transitions):
    from concourse import bass_utils

    feats = np.asarray(features, dtype=F32)
    tags = np.asarray(tag_seqs)
    maskf = np.asarray(mask).astype(F32)
    trans_np = np.asarray(transitions, dtype=F32)

    nc = _get_compiled()
    ep2_bf16 = _host_consts(trans_np)

    in_maps, tstars = [], []
    for c in range(NCORES):
        sl = slice(c * BC, (c + 1) * BC)
        m, ts = _prep_core(feats[sl], tags[sl], maskf[sl], trans_np, ep2_bf16)
        in_maps.append(m)
        tstars.append(ts)

    res = bass_utils.run_bass_kernel_spmd(nc, in_maps, core_ids=list(range(NCORES)))

    s_app = 40 + 32 * np.arange(NEV)               # event ev applied at step s_app
    t_ev = s_app - 9                                # z measured at ring slot 31
    per_seq = []
    for c in range(NCORES):
        out = res.results[c]
        ts = tstars[c]                              # [BC]
        zr = np.asarray(out["zrows"]).astype(np.float64).reshape(T // 16, 16, BC)
        zlast = np.asarray(out["zlast"]).astype(np.float64)[0]
        te = ts + 1                                 # export step of z_{t*}
        bidx = np.arange(BC)
        z_sel = np.where(te >= T, zlast, zr[np.minimum(te // 16, T // 16 - 1),
                                            te % 16, bidx])
        # device divided ef at s_app by the bf16 z exported at step t_ev
        z_ev = zr[t_ev // 16, t_ev % 16, :]         # [NEV, BC]
        applies = s_app[:, None] <= te[None, :]     # export step carries its factor
        logcorr = (np.log(z_ev) * applies).sum(axis=0)
        logZ = np.log(z_sel) + ts * C0 + logcorr
        gold = np.asarray(out["gold"]).astype(np.float64)[:, 0]
        per_seq.append(gold - logZ)

    loss = -np.mean(np.concatenate(per_seq))
    return np.float32(loss)
